# revision 10
# baseline (speedup 1.0000x reference)
"""Trainium2 kernel for nn_AttentionModel_PCA (embedding_lookup).

Math: with sf = softmax(Q^T K) per head,
  G[i,m,a] = sum_h sum_j sf[h,i,j] * V[h,a,Z2[j,m]]
           = sum_{(j,c)} T[(j,c),(i,a)] * E[(j,c),m]
where T[(j,c),(i,a)] = sum_h sf[h,i,j] V[h,a,c]  (tiny H=8 contraction)
and E is the one-hot expansion of Z2. The (5376 x 5376) @ (5376 x M)
GEMM producing G is the dominant cost and runs on the 8 NeuronCores
with M sharded (512 samples per core).

The GEMM runs in fp8e4 DoubleRow perf mode (0.5 PE cycles per output
row vs 4 for fp32): T is scaled by a power-of-2 s into fp8's sweet
spot and E's one-hot 1.0s are exact in fp8; PSUM accumulates fp32 and
the host divides G by s. Final tolerance is 2e-2; fp8 quantization of
T contributes ~0.2% after averaging over the 256-term sums.

Layout: contraction k = t2*256 + two*128 + kp (21 DoubleRow pairs of
128-partition tiles). Stationary weights for output tile pt are
Tp[pt] = [128 kp, 21 t2, 2 two, 128 op] fp8, host-packed so each
per-pt slab load is one contiguous 672KB DMA (5376B per partition).
E sits resident in SBUF as [128 kp, 21 t2, 2 two, 512 m] fp8.
"""

import sys

import numpy as np

for _p in ("/opt/trn_rl_repo",):
    if _p not in sys.path:
        sys.path.append(_p)

H, d, N1, N2, q1, q2, M = 8, 64, 256, 256, 21, 21, 4096
NCORES = 8
MS = M // NCORES          # 512 samples per core
KDIM = N2 * q2            # 5376 contraction (j,c)
PDIM = N1 * q1            # 5376 output rows (i,a)
KT2 = KDIM // 256         # 21 DoubleRow contraction pair-tiles
PT = PDIM // 128          # 42 output-row tiles
LAMBD = 0.001

_PROGRAM = None


def _build_program():
    """Raw bass pipeline: explicit standalone wait_ge + then_inc.

    sync  : 42 per-pt stationary slab loads (3-slot ring, gated by PE)
    scalar: E loaded in 21 per-pair chunks, then G stores
    tensor: 42*21 fp8 DoubleRow matmuls, 2 PSUM banks ping-pong by pt
    vector: PSUM -> SBUF copies (4-slot out ring, gated by stores)
    """
    import concourse.bass as bass
    import concourse.mybir as mybir

    nc = bass.Bass()
    f32 = mybir.dt.float32
    f8 = mybir.dt.float8e4
    Tp = nc.declare_dram_parameter("Tp", [PT, 128, KT2, 2, 128], f8,
                                   isOutput=False)
    E = nc.declare_dram_parameter("E", [KT2, 128, 2 * MS], f8, isOutput=False)
    G = nc.declare_dram_parameter("G", [PDIM, MS], f32, isOutput=True)

    NS = 3                   # slab ring slots
    NOT = 4                  # output ring slots

    with (
        nc.sbuf_tensor([128, KT2, 2, MS], f8) as E_sb,
        nc.sbuf_tensor([128, NS, KT2, 2, 128], f8) as slab,
        nc.sbuf_tensor([128, NOT, MS], f32) as ot,
        nc.psum_tensor([128, 2 * MS], f32) as acc,
        nc.semaphore("e_sem") as e_sem,
        nc.semaphore("dma_sem") as dma_sem,
        nc.semaphore("pe_cnt") as pe_cnt,      # pt fully accumulated by PE
        nc.semaphore("cp_sem") as cp_sem,      # PSUM->SBUF copies done
        nc.semaphore("st_sem") as st_sem,      # G stores done
        nc.Block() as block,
    ):

        @block.sync
        def _(sync):
            for pt in range(PT):
                if pt >= NS:
                    sync.wait_ge(pe_cnt, pt - NS + 1)
                sync.dma_start(slab[:, pt % NS], Tp[pt]).then_inc(dma_sem, 16)

        @block.tensor
        def _(tensor):
            for pt in range(PT):
                half = (pt % 2) * MS
                tensor.wait_ge(dma_sem, 16 * (pt + 1))
                if pt >= 2:
                    # bank reused from pt-2: wait for its copy
                    tensor.wait_ge(cp_sem, pt - 1)
                for t2 in range(KT2):
                    if pt == 0 and t2 == 0:
                        tensor.wait_ge(e_sem, 16)
                    nc.tensor.matmul(
                        acc[:, half:half + MS],
                        slab[:, pt % NS, t2],
                        E_sb[:, t2],
                        start=(t2 == 0),
                        stop=(t2 == KT2 - 1),
                        perf_mode=mybir.MatmulPerfMode.DoubleRow,
                    )
                # drain: PE retires matmuls before the last PSUM writes
                # land; inc pe_cnt only once writeback is done
                tensor.drain().then_inc(pe_cnt, 1)

        @block.vector
        def _(vector):
            for pt in range(PT):
                half = (pt % 2) * MS
                vector.wait_ge(pe_cnt, pt + 1)
                if pt >= NOT:
                    vector.wait_ge(st_sem, 16 * (pt - NOT + 1))
                nc.vector.tensor_copy(
                    ot[:, pt % NOT], acc[:, half:half + MS]
                )
                vector.drain().then_inc(cp_sem, 1)

        @block.scalar
        def _(scalar):
            scalar.dma_start(
                E_sb[:], E.rearrange("t p m -> p t m")
            ).then_inc(e_sem, 16)
            for pt in range(PT):
                scalar.wait_ge(cp_sem, pt + 1)
                scalar.dma_start(
                    G[pt * 128:(pt + 1) * 128, :], ot[:, pt % NOT]
                ).then_inc(st_sem, 16)

    return nc


def host_prep(Q, K, V, Z2):
    """softmax, packed fp8 T (scaled by power-of-2 s), one-hot E (fp8)."""
    import ml_dtypes

    f8 = ml_dtypes.float8_e4m3

    e = np.einsum("hdi,hdj->hij", Q, K, optimize=True)
    e -= e.max(axis=2, keepdims=True)
    np.exp(e, out=e)
    sf = e / e.sum(axis=2, keepdims=True)
    Tt = np.einsum("hij,hac->jcia", sf, V, optimize=True).reshape(KDIM, PDIM)
    Tt = np.ascontiguousarray(Tt, np.float32)

    s = float(2.0 ** np.floor(np.log2(120.0 / max(np.abs(Tt).max(), 1e-30))))
    # [k, p] -> [t2, two, kp, pt, op] -> [pt, kp, t2, two, op]
    Tp = (Tt * s).reshape(KT2, 2, 128, PT, 128).transpose(3, 2, 0, 1, 4)
    Tp = np.ascontiguousarray(Tp).astype(f8)

    Mloc = Z2.shape[1]
    Eoh = np.zeros((KDIM, Mloc), f8)
    rows = (np.arange(N2, dtype=np.int64)[:, None] * q2 + Z2.astype(np.int64))
    Eoh[rows, np.arange(Mloc, dtype=np.int64)[None, :]] = 1.0
    return sf, Tt, (Tp, s), Eoh


def host_tail(G, sf, V, Z1, weights):
    """take_along_axis + logsumexp + loss + regularizer on (N1, M, q1) G."""
    Z1i = Z1.astype(np.int64)
    mat_ene_sum = np.take_along_axis(G, Z1i[:, :, None], axis=2)[..., 0].sum(axis=0)

    Gm = G.max(axis=0)                                   # (M, q1)
    L = np.log(np.exp(G - Gm).sum(axis=0)) + Gm          # (M, q1)
    mx = np.maximum(L.max(axis=1), 0.0)
    logZ = np.log(np.exp(L - mx[:, None]).sum(axis=1)
                  + (N1 - q1) * np.exp(-mx)) + mx

    pl = -(weights.astype(np.float64)
           * (mat_ene_sum.astype(np.float64) - logZ.astype(np.float64))).sum()

    sf2 = sf.reshape(H, -1).astype(np.float64)
    VV = V.reshape(H, -1).astype(np.float64)
    reg = LAMBD * ((sf2 @ sf2.T) * (VV @ VV.T)).sum()
    return np.array(pl + reg, dtype=np.float32)


def run_device(Tp_s, Eoh, trace=False, **kw):
    from concourse.bass_utils import run_bass_kernel_spmd

    Tp, s = Tp_s
    global _PROGRAM
    if _PROGRAM is None:
        _PROGRAM = _build_program()
    # E chunk layout per core: [t2, kp, two*m]
    in_maps = []
    for c in range(NCORES):
        Ec = Eoh[:, c * MS:(c + 1) * MS]
        Ep = np.ascontiguousarray(
            Ec.reshape(KT2, 2, 128, MS).transpose(0, 2, 1, 3)
        ).reshape(KT2, 128, 2 * MS)
        in_maps.append({"Tp": Tp, "E": Ep})
    out = run_bass_kernel_spmd(_PROGRAM, in_maps, list(range(NCORES)),
                               trace=trace, **kw)
    Gf = np.concatenate([np.asarray(out.results[c]["G"]) for c in range(NCORES)],
                        axis=1).astype(np.float32)        # (PDIM, M)
    Gf /= s
    return Gf, out


def kernel(**inputs):
    Q = np.asarray(inputs["Q"], np.float32)
    K = np.asarray(inputs["K"], np.float32)
    V = np.asarray(inputs["V"], np.float32)
    Z1 = np.asarray(inputs["Z1"])
    Z2 = np.asarray(inputs["Z2"])
    weights = np.asarray(inputs["weights"], np.float32)

    sf, _Tt, Tp_s, Eoh = host_prep(Q, K, V, Z2)
    Gf, _ = run_device(Tp_s, Eoh)
    G = Gf.reshape(N1, q1, M).transpose(0, 2, 1)         # (N1, M, q1)
    return host_tail(G, sf, V, Z1, weights)


# revision 15
# speedup vs baseline: 1.0890x; 1.0890x over previous
"""Trainium2 kernel for nn_AttentionModel_PCA (embedding_lookup).

Math: with sf = softmax(Q^T K) per head,
  G[i,m,a] = sum_h sum_j sf[h,i,j] * V[h,a,Z2[j,m]]
           = sum_{(j,c)} T[(j,c),(i,a)] * E[(j,c),m]
where T[(j,c),(i,a)] = sum_h sf[h,i,j] V[h,a,c]  (tiny H=8 contraction)
and E is the one-hot expansion of Z2. The (5376 x 5376) @ (5376 x M)
GEMM producing G is the dominant cost and runs on the 8 NeuronCores
with M sharded (512 samples per core).

The GEMM runs in fp8e4 DoubleRow perf mode (0.5 PE cycles per output
row vs 4 for fp32): T is scaled by a power-of-2 s into fp8's sweet
spot and E's one-hot 1.0s are exact in fp8; PSUM accumulates fp32 and
the host divides G by s. Final tolerance is 2e-2; fp8 quantization of
T contributes ~0.2% after averaging over the 256-term sums.

Layout: contraction k = t2*256 + two*128 + kp (21 DoubleRow pairs of
128-partition tiles). Stationary weights for output tile pt are
Tp[pt] = [128 kp, 21 t2, 2 two, 128 op] fp8, host-packed so each
per-pt slab load is one contiguous 672KB DMA (5376B per partition).
E sits resident in SBUF as [128 kp, 21 t2, 2 two, 512 m] fp8.
"""

import sys

import numpy as np

for _p in ("/opt/trn_rl_repo",):
    if _p not in sys.path:
        sys.path.append(_p)

H, d, N1, N2, q1, q2, M = 8, 64, 256, 256, 21, 21, 4096
NCORES = 8
MS = M // NCORES          # 512 samples per core
KDIM = N2 * q2            # 5376 contraction (j,c)
PDIM = N1 * q1            # 5376 output rows (i,a)
KT2 = KDIM // 256         # 21 DoubleRow contraction pair-tiles
PT = PDIM // 128          # 42 output-row tiles
LAMBD = 0.001

_PROGRAM = None


def _build_program():
    """Raw bass pipeline: explicit standalone wait_ge + then_inc.

    sync  : 42 per-pt stationary slab loads (3-slot ring, gated by PE)
    scalar: E loaded in 21 per-pair chunks, then G stores
    tensor: 42*21 fp8 DoubleRow matmuls, 2 PSUM banks ping-pong by pt
    vector: PSUM -> SBUF copies (4-slot out ring, gated by stores)
    """
    import concourse.bass as bass
    import concourse.mybir as mybir

    nc = bass.Bass()
    f32 = mybir.dt.float32
    f8 = mybir.dt.float8e4
    Tp = nc.declare_dram_parameter("Tp", [PT, 128, KT2, 2, 128], f8,
                                   isOutput=False)
    E = nc.declare_dram_parameter("E", [KT2, 128, 2 * MS], f8, isOutput=False)
    G = nc.declare_dram_parameter("G", [PDIM, MS], f32, isOutput=True)

    NS = 3                   # slab ring slots
    NOT = 4                  # output ring slots

    with (
        nc.sbuf_tensor([128, KT2, 2, MS], f8) as E_sb,
        nc.sbuf_tensor([128, NS, KT2, 2, 128], f8) as slab,
        nc.sbuf_tensor([128, NOT, MS], f32) as ot,
        nc.psum_tensor([128, 2 * MS], f32) as acc,
        nc.semaphore("e_sem") as e_sem,
        nc.semaphore("sl_sem0") as sl_sem0,
        nc.semaphore("sl_sem1") as sl_sem1,
        nc.semaphore("sl_sem2") as sl_sem2,
        nc.semaphore("pe_cnt") as pe_cnt,      # pt fully accumulated by PE
        nc.semaphore("cp_sem") as cp_sem,      # PSUM->SBUF copies done
        nc.semaphore("st_sem0") as st_sem0,
        nc.semaphore("st_sem1") as st_sem1,
        nc.semaphore("st_sem2") as st_sem2,
        nc.semaphore("st_sem3") as st_sem3,
        nc.Block() as block,
    ):
        # per-slot ring semaphores: DMA completions within one queue can
        # finish out of order across the 16 DMA engines, so cumulative
        # counts on a shared semaphore can pass before the specific
        # transfer being waited on is done. Slot reuse is consumption-
        # gated, so each slot's counter only sees its own transfer.
        sl_sems = [sl_sem0, sl_sem1, sl_sem2]
        st_sems = [st_sem0, st_sem1, st_sem2, st_sem3]

        @block.sync
        def _(sync):
            for pt in range(PT):
                if pt >= NS:
                    sync.wait_ge(pe_cnt, pt - NS + 1)
                sync.dma_start(slab[:, pt % NS], Tp[pt]) \
                    .then_inc(sl_sems[pt % NS], 16)

        @block.tensor
        def _(tensor):
            for pt in range(PT):
                half = (pt % 2) * MS
                tensor.wait_ge(sl_sems[pt % NS], 16 * (pt // NS + 1))
                if pt >= 2:
                    # bank reused from pt-2: wait for its copy
                    tensor.wait_ge(cp_sem, pt - 1)
                for t2 in range(KT2):
                    if pt == 0 and t2 == 0:
                        tensor.wait_ge(e_sem, 16)
                    mm = nc.tensor.matmul(
                        acc[:, half:half + MS],
                        slab[:, pt % NS, t2],
                        E_sb[:, t2],
                        start=(t2 == 0),
                        stop=(t2 == KT2 - 1),
                        perf_mode=mybir.MatmulPerfMode.DoubleRow,
                    )
                    if t2 == KT2 - 1:
                        mm.then_inc(pe_cnt, 1)
            # PE retires matmuls before their last PSUM writes land; the
            # copier waits one extra pt of settle, and this drained inc
            # covers the final pt (no successor exists)
            tensor.drain().then_inc(pe_cnt, 1)

        @block.vector
        def _(vector):
            for pt in range(PT):
                half = (pt % 2) * MS
                vector.wait_ge(pe_cnt, pt + 2)
                if pt >= NOT:
                    vector.wait_ge(st_sems[pt % NOT], 16 * (pt // NOT))
                nc.vector.tensor_copy(
                    ot[:, pt % NOT], acc[:, half:half + MS]
                )
                vector.drain().then_inc(cp_sem, 1)

        @block.scalar
        def _(scalar):
            scalar.dma_start(
                E_sb[:], E.rearrange("t p m -> p t m")
            ).then_inc(e_sem, 16)
            for pt in range(PT):
                scalar.wait_ge(cp_sem, pt + 1)
                scalar.dma_start(
                    G[pt * 128:(pt + 1) * 128, :], ot[:, pt % NOT]
                ).then_inc(st_sems[pt % NOT], 16)

    return nc


def host_prep(Q, K, V, Z2):
    """softmax, packed fp8 T (scaled by power-of-2 s), one-hot E (fp8)."""
    import ml_dtypes

    f8 = ml_dtypes.float8_e4m3

    e = np.einsum("hdi,hdj->hij", Q, K, optimize=True)
    e -= e.max(axis=2, keepdims=True)
    np.exp(e, out=e)
    sf = e / e.sum(axis=2, keepdims=True)
    Tt = np.einsum("hij,hac->jcia", sf, V, optimize=True).reshape(KDIM, PDIM)
    Tt = np.ascontiguousarray(Tt, np.float32)

    s = float(2.0 ** np.floor(np.log2(120.0 / max(np.abs(Tt).max(), 1e-30))))
    # [k, p] -> [t2, two, kp, pt, op] -> [pt, kp, t2, two, op]
    Tp = (Tt * s).reshape(KT2, 2, 128, PT, 128).transpose(3, 2, 0, 1, 4)
    Tp = np.ascontiguousarray(Tp).astype(f8)

    Mloc = Z2.shape[1]
    Eoh = np.zeros((KDIM, Mloc), f8)
    rows = (np.arange(N2, dtype=np.int64)[:, None] * q2 + Z2.astype(np.int64))
    Eoh[rows, np.arange(Mloc, dtype=np.int64)[None, :]] = 1.0
    return sf, Tt, (Tp, s), Eoh


def host_tail(G, sf, V, Z1, weights):
    """take_along_axis + logsumexp + loss + regularizer on (N1, M, q1) G."""
    Z1i = Z1.astype(np.int64)
    mat_ene_sum = np.take_along_axis(G, Z1i[:, :, None], axis=2)[..., 0].sum(axis=0)

    Gm = G.max(axis=0)                                   # (M, q1)
    L = np.log(np.exp(G - Gm).sum(axis=0)) + Gm          # (M, q1)
    mx = np.maximum(L.max(axis=1), 0.0)
    logZ = np.log(np.exp(L - mx[:, None]).sum(axis=1)
                  + (N1 - q1) * np.exp(-mx)) + mx

    pl = -(weights.astype(np.float64)
           * (mat_ene_sum.astype(np.float64) - logZ.astype(np.float64))).sum()

    sf2 = sf.reshape(H, -1).astype(np.float64)
    VV = V.reshape(H, -1).astype(np.float64)
    reg = LAMBD * ((sf2 @ sf2.T) * (VV @ VV.T)).sum()
    return np.array(pl + reg, dtype=np.float32)


def run_device(Tp_s, Eoh, trace=False, **kw):
    from concourse.bass_utils import run_bass_kernel_spmd

    Tp, s = Tp_s
    global _PROGRAM
    if _PROGRAM is None:
        _PROGRAM = _build_program()
    # E chunk layout per core: [t2, kp, two*m]
    in_maps = []
    for c in range(NCORES):
        Ec = Eoh[:, c * MS:(c + 1) * MS]
        Ep = np.ascontiguousarray(
            Ec.reshape(KT2, 2, 128, MS).transpose(0, 2, 1, 3)
        ).reshape(KT2, 128, 2 * MS)
        in_maps.append({"Tp": Tp, "E": Ep})
    out = run_bass_kernel_spmd(_PROGRAM, in_maps, list(range(NCORES)),
                               trace=trace, **kw)
    Gf = np.concatenate([np.asarray(out.results[c]["G"]) for c in range(NCORES)],
                        axis=1).astype(np.float32)        # (PDIM, M)
    Gf /= s
    return Gf, out


def kernel(**inputs):
    Q = np.asarray(inputs["Q"], np.float32)
    K = np.asarray(inputs["K"], np.float32)
    V = np.asarray(inputs["V"], np.float32)
    Z1 = np.asarray(inputs["Z1"])
    Z2 = np.asarray(inputs["Z2"])
    weights = np.asarray(inputs["weights"], np.float32)

    sf, _Tt, Tp_s, Eoh = host_prep(Q, K, V, Z2)
    Gf, _ = run_device(Tp_s, Eoh)
    G = Gf.reshape(N1, q1, M).transpose(0, 2, 1)         # (N1, M, q1)
    return host_tail(G, sf, V, Z1, weights)


# revision 17
# speedup vs baseline: 1.1526x; 1.0584x over previous
"""Trainium2 kernel for nn_AttentionModel_PCA (embedding_lookup).

Math: with sf = softmax(Q^T K) per head,
  G[i,m,a] = sum_h sum_j sf[h,i,j] * V[h,a,Z2[j,m]]
           = sum_{(j,c)} T[(j,c),(i,a)] * E[(j,c),m]
where T[(j,c),(i,a)] = sum_h sf[h,i,j] V[h,a,c]  (tiny H=8 contraction)
and E is the one-hot expansion of Z2. The (5376 x 5376) @ (5376 x M)
GEMM producing G is the dominant cost and runs on the 8 NeuronCores
with M sharded (512 samples per core).

The GEMM runs in fp8e4 DoubleRow perf mode (0.5 PE cycles per output
row vs 4 for fp32): T is scaled by a power-of-2 s into fp8's sweet
spot and E's one-hot 1.0s are exact in fp8; PSUM accumulates fp32 and
the host divides G by s. Final tolerance is 2e-2; fp8 quantization of
T contributes ~0.2% after averaging over the 256-term sums.

Layout: contraction k = t2*256 + two*128 + kp (21 DoubleRow pairs of
128-partition tiles). Stationary weights for output tile pt are
Tp[pt] = [128 kp, 21 t2, 2 two, 128 op] fp8, host-packed so each
per-pt slab load is one contiguous 672KB DMA (5376B per partition).
E sits resident in SBUF as [128 kp, 21 t2, 2 two, 512 m] fp8.
"""

import sys

import numpy as np

for _p in ("/opt/trn_rl_repo",):
    if _p not in sys.path:
        sys.path.append(_p)

H, d, N1, N2, q1, q2, M = 8, 64, 256, 256, 21, 21, 4096
NCORES = 8
MS = M // NCORES          # 512 samples per core
KDIM = N2 * q2            # 5376 contraction (j,c)
PDIM = N1 * q1            # 5376 output rows (i,a)
KT2 = KDIM // 256         # 21 DoubleRow contraction pair-tiles
PT = PDIM // 128          # 42 output-row tiles
LAMBD = 0.001

_PROGRAM = None


def _build_program():
    """Raw bass pipeline: explicit standalone wait_ge + then_inc.

    sync  : 42 per-pt stationary slab loads (3-slot ring, gated by PE)
    scalar: E loaded in 21 per-pair chunks, then G stores
    tensor: 42*21 fp8 DoubleRow matmuls, 2 PSUM banks ping-pong by pt
    vector: PSUM -> SBUF copies (4-slot out ring, gated by stores)
    """
    import concourse.bass as bass
    import concourse.mybir as mybir

    nc = bass.Bass()
    f32 = mybir.dt.float32
    f8 = mybir.dt.float8e4
    Tp = nc.declare_dram_parameter("Tp", [PT, 128, KT2, 2, 128], f8,
                                   isOutput=False)
    E = nc.declare_dram_parameter("E", [KT2, 128, 2 * MS], f8, isOutput=False)
    G = nc.declare_dram_parameter("G", [PDIM, MS], f32, isOutput=True)

    NS = 3                   # slab ring slots
    NOT = 4                  # output ring slots

    with (
        nc.sbuf_tensor([128, KT2, 2, MS], f8) as E_sb,
        nc.sbuf_tensor([128, NS, KT2, 2, 128], f8) as slab,
        nc.sbuf_tensor([128, NOT, MS], f32) as ot,
        nc.psum_tensor([128, 2 * MS], f32) as acc,
        nc.semaphore("e_sem") as e_sem,
        nc.semaphore("sl_sem0") as sl_sem0,
        nc.semaphore("sl_sem1") as sl_sem1,
        nc.semaphore("sl_sem2") as sl_sem2,
        nc.semaphore("pe_cnt") as pe_cnt,      # pt fully accumulated by PE
        nc.semaphore("cp_sem") as cp_sem,      # PSUM->SBUF copies done
        nc.semaphore("st_sem0") as st_sem0,
        nc.semaphore("st_sem1") as st_sem1,
        nc.semaphore("st_sem2") as st_sem2,
        nc.semaphore("st_sem3") as st_sem3,
        nc.Block() as block,
    ):
        # per-slot ring semaphores: DMA completions within one queue can
        # finish out of order across the 16 DMA engines, so cumulative
        # counts on a shared semaphore can pass before the specific
        # transfer being waited on is done. Slot reuse is consumption-
        # gated, so each slot's counter only sees its own transfer.
        sl_sems = [sl_sem0, sl_sem1, sl_sem2]
        st_sems = [st_sem0, st_sem1, st_sem2, st_sem3]

        @block.sync
        def _(sync):
            for pt in range(PT):
                if pt >= NS:
                    sync.wait_ge(pe_cnt, pt - NS + 1)
                sync.dma_start(slab[:, pt % NS], Tp[pt]) \
                    .then_inc(sl_sems[pt % NS], 16)

        @block.tensor
        def _(tensor):
            for pt in range(PT):
                half = (pt % 2) * MS
                tensor.wait_ge(sl_sems[pt % NS], 16 * (pt // NS + 1))
                if pt >= 2:
                    # bank reused from pt-2: wait for its copy
                    tensor.wait_ge(cp_sem, pt - 1)
                for t2 in range(KT2):
                    if pt == 0 and t2 == 0:
                        tensor.wait_ge(e_sem, 16)
                    nc.tensor.matmul(
                        acc[:, half:half + MS],
                        slab[:, pt % NS, t2],
                        E_sb[:, t2],
                        start=(t2 == 0),
                        stop=(t2 == KT2 - 1),
                        perf_mode=mybir.MatmulPerfMode.DoubleRow,
                    )
                # the PE's semaphore updates run ahead of the array (and
                # PSUM writeback), so a plain matmul.then_inc is not a safe
                # completion signal; drain once per pt. pe_cnt counts
                # fully-executed pts (slab consumed, PSUM writes landed).
                tensor.drain().then_inc(pe_cnt, 1)

        @block.vector
        def _(vector):
            for pt in range(PT):
                half = (pt % 2) * MS
                vector.wait_ge(pe_cnt, pt + 1)
                if pt >= NOT:
                    vector.wait_ge(st_sems[pt % NOT], 16 * (pt // NOT))
                nc.vector.tensor_copy(
                    ot[:, pt % NOT], acc[:, half:half + MS]
                )
                vector.drain().then_inc(cp_sem, 1)

        @block.scalar
        def _(scalar):
            scalar.dma_start(
                E_sb[:], E.rearrange("t p m -> p t m")
            ).then_inc(e_sem, 16)
            for pt in range(PT):
                scalar.wait_ge(cp_sem, pt + 1)
                scalar.dma_start(
                    G[pt * 128:(pt + 1) * 128, :], ot[:, pt % NOT]
                ).then_inc(st_sems[pt % NOT], 16)

    return nc


def host_prep(Q, K, V, Z2):
    """softmax, packed fp8 T (scaled by power-of-2 s), one-hot E (fp8)."""
    import ml_dtypes

    f8 = ml_dtypes.float8_e4m3

    e = np.einsum("hdi,hdj->hij", Q, K, optimize=True)
    e -= e.max(axis=2, keepdims=True)
    np.exp(e, out=e)
    sf = e / e.sum(axis=2, keepdims=True)
    Tt = np.einsum("hij,hac->jcia", sf, V, optimize=True).reshape(KDIM, PDIM)
    Tt = np.ascontiguousarray(Tt, np.float32)

    s = float(2.0 ** np.floor(np.log2(120.0 / max(np.abs(Tt).max(), 1e-30))))
    # [k, p] -> [t2, two, kp, pt, op] -> [pt, kp, t2, two, op]
    Tp = (Tt * s).reshape(KT2, 2, 128, PT, 128).transpose(3, 2, 0, 1, 4)
    Tp = np.ascontiguousarray(Tp).astype(f8)

    Mloc = Z2.shape[1]
    Eoh = np.zeros((KDIM, Mloc), f8)
    rows = (np.arange(N2, dtype=np.int64)[:, None] * q2 + Z2.astype(np.int64))
    Eoh[rows, np.arange(Mloc, dtype=np.int64)[None, :]] = 1.0
    return sf, Tt, (Tp, s), Eoh


def host_tail(G, sf, V, Z1, weights):
    """take_along_axis + logsumexp + loss + regularizer on (N1, M, q1) G."""
    Z1i = Z1.astype(np.int64)
    mat_ene_sum = np.take_along_axis(G, Z1i[:, :, None], axis=2)[..., 0].sum(axis=0)

    Gm = G.max(axis=0)                                   # (M, q1)
    L = np.log(np.exp(G - Gm).sum(axis=0)) + Gm          # (M, q1)
    mx = np.maximum(L.max(axis=1), 0.0)
    logZ = np.log(np.exp(L - mx[:, None]).sum(axis=1)
                  + (N1 - q1) * np.exp(-mx)) + mx

    pl = -(weights.astype(np.float64)
           * (mat_ene_sum.astype(np.float64) - logZ.astype(np.float64))).sum()

    sf2 = sf.reshape(H, -1).astype(np.float64)
    VV = V.reshape(H, -1).astype(np.float64)
    reg = LAMBD * ((sf2 @ sf2.T) * (VV @ VV.T)).sum()
    return np.array(pl + reg, dtype=np.float32)


def run_device(Tp_s, Eoh, trace=False, **kw):
    from concourse.bass_utils import run_bass_kernel_spmd

    Tp, s = Tp_s
    global _PROGRAM
    if _PROGRAM is None:
        _PROGRAM = _build_program()
    # E chunk layout per core: [t2, kp, two*m]
    in_maps = []
    for c in range(NCORES):
        Ec = Eoh[:, c * MS:(c + 1) * MS]
        Ep = np.ascontiguousarray(
            Ec.reshape(KT2, 2, 128, MS).transpose(0, 2, 1, 3)
        ).reshape(KT2, 128, 2 * MS)
        in_maps.append({"Tp": Tp, "E": Ep})
    out = run_bass_kernel_spmd(_PROGRAM, in_maps, list(range(NCORES)),
                               trace=trace, **kw)
    Gf = np.concatenate([np.asarray(out.results[c]["G"]) for c in range(NCORES)],
                        axis=1).astype(np.float32)        # (PDIM, M)
    Gf /= s
    return Gf, out


def kernel(**inputs):
    Q = np.asarray(inputs["Q"], np.float32)
    K = np.asarray(inputs["K"], np.float32)
    V = np.asarray(inputs["V"], np.float32)
    Z1 = np.asarray(inputs["Z1"])
    Z2 = np.asarray(inputs["Z2"])
    weights = np.asarray(inputs["weights"], np.float32)

    sf, _Tt, Tp_s, Eoh = host_prep(Q, K, V, Z2)
    Gf, _ = run_device(Tp_s, Eoh)
    G = Gf.reshape(N1, q1, M).transpose(0, 2, 1)         # (N1, M, q1)
    return host_tail(G, sf, V, Z1, weights)


# revision 19
# speedup vs baseline: 1.1696x; 1.0148x over previous
"""Trainium2 kernel for nn_AttentionModel_PCA (embedding_lookup).

Exploits the H-factorization to shrink the GEMM contraction from 5376
(one-hot (j,c) rows) to 2048 ((h,j) rows):

  G[i,a,m] = sum_{h,j} sf[h,i,j] * W[(h,j),(a,m)],
  W[(h,j),(a,m)] = V[h,a,Z2[j,m]]   (host-gathered, fp8)

Per core (M sharded 8 ways, MS=512): 42 chains (2 i-tiles x 21 a) of 8
accumulating fp8 DoubleRow matmuls -> 336 matmuls instead of 882 for
the one-hot formulation. sf (x64) and V (x16) are quantized to fp8e4;
PSUM accumulates fp32; host divides G by 1024.

W streams in 21 per-a chunks (8KB/partition contiguous each) through a
6-slot ring; sf stationaries are SBUF-resident; 8 PSUM banks round-robin
the chains.
"""

import sys

import numpy as np

for _p in ("/opt/trn_rl_repo",):
    if _p not in sys.path:
        sys.path.append(_p)

H, d, N1, N2, q1, q2, M = 8, 64, 256, 256, 21, 21, 4096
NCORES = 8
MS = M // NCORES          # 512 samples per core
HJ = H * N2               # 2048 contraction rows (h,j)
PTI = N1 // 128           # 2 i-tiles
SF_SCALE = 64.0
V_SCALE = 16.0
LAMBD = 0.001

_PROGRAM = None


def _build_program():
    """sync: 21 per-a W chunk loads (6-slot ring, copy-gated reuse)
    scalar: sf stationary load, then G stores
    tensor: 42 chains x 8 fp8 DoubleRow matmuls, 8 PSUM banks round-robin,
            one PE drain per chunk as the completion signal
    vector: PSUM -> SBUF copies (4-slot out ring, gated by stores)
    """
    import concourse.bass as bass
    import concourse.mybir as mybir

    nc = bass.Bass()
    f32 = mybir.dt.float32
    f8 = mybir.dt.float8e4
    SF = nc.declare_dram_parameter("SF", [H, 128, 2, N1], f8, isOutput=False)
    # W split into two params: single >16MiB transfers showed intermittent
    # corruption around the 10MiB mark on the transfer path
    NW1 = 11
    WA = nc.declare_dram_parameter("WA", [NW1, 128, H, 2, MS], f8,
                                   isOutput=False)
    WB = nc.declare_dram_parameter("WB", [q1 - NW1, 128, H, 2, MS], f8,
                                   isOutput=False)
    G = nc.declare_dram_parameter("G", [PTI * q1 * 128, MS], f32, isOutput=True)

    NWS = 6                  # W ring slots
    NOT = 4                  # output ring slots
    NCHAIN = PTI * q1        # 42

    with (
        nc.sbuf_tensor([128, H, 2, N1], f8) as sf_sb,
        nc.sbuf_tensor([128, NWS, H, 2, MS], f8) as w_sb,
        nc.sbuf_tensor([128, NOT, MS], f32) as ot,
        nc.psum_tensor([128, 8 * MS], f32) as acc,
        nc.semaphore("sf_sem") as sf_sem,
        nc.semaphore("w_sem0") as w_sem0,
        nc.semaphore("w_sem1") as w_sem1,
        nc.semaphore("w_sem2") as w_sem2,
        nc.semaphore("w_sem3") as w_sem3,
        nc.semaphore("w_sem4") as w_sem4,
        nc.semaphore("w_sem5") as w_sem5,
        nc.semaphore("pe_cnt") as pe_cnt,      # chains retired by PE
        nc.semaphore("cp_sem") as cp_sem,      # PSUM->SBUF copies done
        nc.semaphore("st_sem0") as st_sem0,
        nc.semaphore("st_sem1") as st_sem1,
        nc.semaphore("st_sem2") as st_sem2,
        nc.semaphore("st_sem3") as st_sem3,
        nc.Block() as block,
    ):
        # per-slot W semaphores: slot reuse is consumption-gated, so each
        # slot's counter can only be advanced by the chunk the PE is about
        # to wait for (no cross-chunk engine-split pollution)
        w_sems = [w_sem0, w_sem1, w_sem2, w_sem3, w_sem4, w_sem5]

        @block.sync
        def _(sync):
            for a in range(q1):
                if a >= NWS:
                    # slot reused from chunk a-NWS: gate on the PSUM->SBUF
                    # copies of both of its chains (the copies execute
                    # strictly after the chains consumed the slot, adding
                    # margin beyond pe_cnt's drain signal)
                    sync.wait_ge(cp_sem, 2 * (a - NWS) + 2)
                src = WA[a] if a < NW1 else WB[a - NW1]
                sync.dma_start(w_sb[:, a % NWS], src) \
                    .then_inc(w_sems[a % NWS], 16)

        @block.tensor
        def _(tensor):
            tensor.wait_ge(sf_sem, 16)
            for a in range(q1):
                tensor.wait_ge(w_sems[a % NWS], 16 * (a // NWS + 1))
                for pt in range(PTI):
                    k = 2 * a + pt
                    bank = k % 8
                    if k >= 8:
                        tensor.wait_ge(cp_sem, k - 7)
                    for h in range(H):
                        nc.tensor.matmul(
                            acc[:, bank * MS:(bank + 1) * MS],
                            sf_sb[:, h, :, pt * 128:(pt + 1) * 128],
                            w_sb[:, a % NWS, h],
                            start=(h == 0),
                            stop=(h == H - 1),
                            perf_mode=mybir.MatmulPerfMode.DoubleRow,
                        )
                # the PE's semaphore updates run ahead of the array (and
                # PSUM writeback), so a plain matmul.then_inc is not a safe
                # completion signal; drain once per chunk. pe_cnt therefore
                # counts fully-executed chunks (both chains, writes landed).
                tensor.drain().then_inc(pe_cnt, 1)

        # Per-slot store semaphores: DMA completions within one queue can
        # finish out of order across the 16 DMA engines, so cumulative
        # counts on a shared semaphore can pass before the specific store
        # being waited on is done. Slot reuse is consumption-gated, so each
        # slot's counter only ever sees the store the waiter cares about.
        ST_INC = 16
        st_sems = [st_sem0, st_sem1, st_sem2, st_sem3]

        @block.vector
        def _(vector):
            for k in range(NCHAIN):
                bank = k % 8
                vector.wait_ge(pe_cnt, k // 2 + 1)
                if k >= NOT:
                    # ot slot reused: store of chain k-NOT must be done
                    vector.wait_ge(st_sems[k % NOT], ST_INC * (k // NOT))
                nc.vector.tensor_copy(
                    ot[:, k % NOT], acc[:, bank * MS:(bank + 1) * MS]
                )
                vector.drain().then_inc(cp_sem, 1)

        @block.scalar
        def _(scalar):
            scalar.dma_start(sf_sb[:], SF.rearrange("h p two i -> p h two i")) \
                .then_inc(sf_sem, 16)
            for k in range(NCHAIN):
                scalar.wait_ge(cp_sem, k + 1)
                scalar.dma_start(
                    G[k * 128:(k + 1) * 128, :], ot[:, k % NOT]
                ).then_inc(st_sems[k % NOT], 16)

    return nc


def host_prep(Q, K, V, Z2):
    """softmax; fp8 packed sf stationaries and gathered W."""
    import ml_dtypes

    f8 = ml_dtypes.float8_e4m3

    e = np.einsum("hdi,hdj->hij", Q, K, optimize=True)
    e -= e.max(axis=2, keepdims=True)
    np.exp(e, out=e)
    sf = e / e.sum(axis=2, keepdims=True)

    # SF[h, kp, two, i] = 64*sf[h, i, j=two*128+kp]
    SFp = np.ascontiguousarray(
        (sf * SF_SCALE).transpose(0, 2, 1)          # (h, j, i)
        .reshape(H, 2, 128, N1)                     # (h, two, kp, i)
        .transpose(0, 2, 1, 3)                      # (h, kp, two, i)
    ).astype(f8)

    # W[a, kp, h, two, m] = 16*V[h, a, Z2[two*128+kp, m]]
    Vq = (V * V_SCALE).astype(f8)
    Z2i = Z2.astype(np.int64)
    Vsel = Vq[:, :, Z2i]                            # (h, a, j, m) fp8
    Mloc = Z2.shape[1]
    Wp = np.ascontiguousarray(
        Vsel.reshape(H, q1, 2, 128, Mloc)           # (h, a, two, kp, m)
        .transpose(1, 3, 0, 2, 4)                   # (a, kp, h, two, m)
    )
    return sf, SFp, Wp


def dequant(SFp, Wp):
    """fp32 views of what the PE sees: sfq (h,i,j), Vsel_q (h,a,j,m)."""
    sfq = (SFp.astype(np.float32)                    # (h, kp, two, i)
           .transpose(0, 2, 1, 3)                    # (h, two, kp, i)
           .reshape(H, N2, N1).transpose(0, 2, 1)) / SF_SCALE
    Mloc = Wp.shape[4]
    vselq = (Wp.astype(np.float32)                   # (a, kp, h, two, m)
             .transpose(2, 0, 3, 1, 4)               # (h, a, two, kp, m)
             .reshape(H, q1, N2, Mloc)) / V_SCALE
    return sfq, vselq


def host_tail(G, sf, V, Z1, weights):
    """take_along_axis + logsumexp + loss + regularizer on (N1, M, q1) G."""
    Z1i = Z1.astype(np.int64)
    mat_ene_sum = np.take_along_axis(G, Z1i[:, :, None], axis=2)[..., 0].sum(axis=0)

    Gm = G.max(axis=0)                                   # (M, q1)
    L = np.log(np.exp(G - Gm).sum(axis=0)) + Gm          # (M, q1)
    mx = np.maximum(L.max(axis=1), 0.0)
    logZ = np.log(np.exp(L - mx[:, None]).sum(axis=1)
                  + (N1 - q1) * np.exp(-mx)) + mx

    pl = -(weights.astype(np.float64)
           * (mat_ene_sum.astype(np.float64) - logZ.astype(np.float64))).sum()

    sf2 = sf.reshape(H, -1).astype(np.float64)
    VV = V.reshape(H, -1).astype(np.float64)
    reg = LAMBD * ((sf2 @ sf2.T) * (VV @ VV.T)).sum()
    return np.array(pl + reg, dtype=np.float32)


def run_device(SFp, Wp, trace=False, **kw):
    from concourse.bass_utils import run_bass_kernel_spmd

    global _PROGRAM
    if _PROGRAM is None:
        _PROGRAM = _build_program()
    in_maps = []
    for c in range(NCORES):
        Wc = np.ascontiguousarray(Wp[..., c * MS:(c + 1) * MS])
        in_maps.append({"SF": SFp, "WA": np.ascontiguousarray(Wc[:11]),
                        "WB": np.ascontiguousarray(Wc[11:])})
    out = run_bass_kernel_spmd(_PROGRAM, in_maps, list(range(NCORES)),
                               trace=trace, **kw)
    # per-core G: [(a, pt, ip), m] rows in chain order k=2a+pt
    Gfull = np.empty((N1, M, q1), np.float32)
    for c in range(NCORES):
        g = np.asarray(out.results[c]["G"]).reshape(q1, PTI, 128, MS)
        Gfull[:, c * MS:(c + 1) * MS, :] = (
            g.transpose(1, 2, 3, 0).reshape(N1, MS, q1))
    Gfull /= (SF_SCALE * V_SCALE)
    return Gfull, out


def kernel(**inputs):
    Q = np.asarray(inputs["Q"], np.float32)
    K = np.asarray(inputs["K"], np.float32)
    V = np.asarray(inputs["V"], np.float32)
    Z1 = np.asarray(inputs["Z1"])
    Z2 = np.asarray(inputs["Z2"])
    weights = np.asarray(inputs["weights"], np.float32)

    sf, SFp, Wp = host_prep(Q, K, V, Z2)
    G, _ = run_device(SFp, Wp)
    return host_tail(G, sf, V, Z1, weights)
